# revision 22
# baseline (speedup 1.0000x reference)
"""GQA attention decode step (B=8, S=16, D=4096, H=32, KVH=8, HD=128) on 8
Trainium2 NeuronCores, tensor-parallel over heads.

Core c owns q-heads [4c, 4c+4), kv-head c, wo rows [512c, 512(c+1)).
Each core computes a full (128, 4096) partial output; the host sums the 8
partials (the all-reduce) and reshapes to (B, S, D).
"""

import os
import sys

import numpy as np

sys.path.insert(0, "/opt/trn_rl_repo")

import concourse.bacc as bacc  # noqa: E402
import concourse.mybir as mybir  # noqa: E402
import concourse.tile as tile  # noqa: E402
from concourse.tile import add_dep_helper  # noqa: E402
from concourse import bass_utils  # noqa: E402

D = 4096
H = 32
KVH = 8
HD = 128
NCORES = 8
HQ = H // NCORES          # 4 q heads per core
CW = HQ * HD              # 512 = per-core q width / wo row count
KVW = 2 * HD              # 256 = fused [k|v] projection width

TRACE = False
# "bf16c": hi/lo bf16 compensated matmuls (f32-equivalent bytes, 3 passes)
# "bf16":  pure bf16 weights/cache/activations (half the DMA bytes, 1 pass)
# "f32":   plain fp32
MODE = "bf16"
USE_BF16C = MODE == "bf16c"
_LAST = None              # last BassKernelResults (for test.py)
_BUILD_CACHE = {}

F32 = mybir.dt.float32


def _build(L, B, S, reps=1, loop_reps=1, skip_in_dma=False):
    """Build the per-core Bass program (identical across cores; SPMD).

    reps>1 replicates the whole body (for on-device benchmarking: one
    dispatch runs the kernel reps times back-to-back).  loop_reps>1
    instead wraps the body in a For_i hardware loop (cheap compile,
    serialized by a per-iteration all-engine barrier)."""
    T = B * S
    assert T == 128, "kernel assumes 128 tokens (B*S)"
    nfull, rem = divmod(L, 128)
    CS = [128] * nfull + ([rem] if rem else [])  # cache kv-chunk sizes
    ncache = len(CS)
    tpb = ncache                                  # cache tiles per batch
    NCHUNK = ncache + 1                           # + new-kv chunk (S wide)
    QW = HQ * S                                   # 64 score columns (4 heads x 16 tok)
    SCW = NCHUNK * QW                             # scoresT psum width per batch
    NKT = D // 128                                # 32 contraction tiles
    WCH = 8                                       # k-tiles per weight DMA chunk
    NWCH = NKT // WCH                             # 4 weight chunks
    scale = 1.0 / float(np.sqrt(HD))

    nc = bacc.Bacc("TRN2", target_bir_lowering=False, debug=False)
    BF = mybir.dt.bfloat16
    WMUL = 2 if USE_BF16C else 1
    WT = F32 if MODE == "f32" else BF

    xt_d = nc.dram_tensor("xt", (128, WMUL * D), WT, kind="ExternalInput")
    wq_d = nc.dram_tensor("wq", (128, WMUL * NKT * CW), WT, kind="ExternalInput")
    wkv_d = nc.dram_tensor("wkv", (128, WMUL * NKT * KVW), WT, kind="ExternalInput")
    wo_d = nc.dram_tensor("wo", (128, WMUL * HQ * D), WT, kind="ExternalInput")
    RT = BF if MODE == "bf16" else F32
    RW = (HQ + 1) * HD + (HQ + 1) * HD // 2   # crep | shalf packed
    # consts slab: crep | shalf | ident in one transfer
    rotc_d = nc.dram_tensor("rotc", (128, RW + 128), RT, kind="ExternalInput")
    if ncache:
        keyst_d = nc.dram_tensor("keyst", (128, WMUL * B * tpb * 128), WT, kind="ExternalInput")
        vals_d = nc.dram_tensor("vals", (128, WMUL * B * tpb * (HD + 1)), WT, kind="ExternalInput")
    OUTT = BF if MODE == "bf16" else F32
    out_d = nc.dram_tensor("out", (T, D), OUTT, kind="ExternalOutput")

    import contextlib

    with tile.TileContext(nc) as tc:
      with (tc.For_i(0, loop_reps) if loop_reps > 1
            else contextlib.nullcontext()):
       for _rep in range(reps):
        with tc.tile_pool(name=f"const{_rep}", bufs=1) as cpool:
            rotc = cpool.tile([128, RW + 128], RT)
            crep = rotc[:, 0:(HQ + 1) * HD]
            shalf = rotc[:, (HQ + 1) * HD:RW]
            ident = rotc[:, RW:RW + 128]
            # long-lived intermediates (all bf16: transposes run 1 cyc/row)
            qtH = cpool.tile([128, CW], BF)            # (hd, b*64 + h*16 + s)
            ktH = cpool.tile([128, 128], BF)           # (hd, tok)
            vrebB = cpool.tile([S, B * (HD + 1)], BF)  # new-v, partition-rebased
            ctxtB = cpool.tile([128, B * QW], BF)      # (hd, h*128 + b*16 + s)
            otile = cpool.tile([128, D], OUTT)         # assembled output

            # persistent kv-cache tiles: allocated up-front (SBUF-disjoint from
            # the transient weight pools) so their DMAs can issue right after
            # the weight stream with no WAR waits
            if ncache:
                KBW = WMUL * 128
                VBW = WMUL * (HD + 1)
                kc_all = cpool.tile([128, B * tpb * KBW], WT, name="kcAll")
                vc_half = [
                    cpool.tile([128, (B // 2) * tpb * VBW], WT, name=f"vcH{h}")
                    for h in range(2)]
            kv_last_inst = None

            # ---------------- phase 1: qkv projections ----------------
            # DMA plan for the real platform: per-core line rate ~375 GB/s
            # (any queue alone can reach it, no multi-core contention), but
            # each dma_start costs ~1us serial on its queue (+2us if dep-
            # chained).  So: FEW, BIG transfers, bytes balanced across the
            # 3 queues, ordered so consumers fire as streams land, and NO
            # dep chains on input streams (emission order is preserved).
            # SP:  xt, wqA x2, wkv, out0123, out4
            # ACT: wqB x2, vc b4-7, wo c5-7, out567
            # Pool: consts, kc b0-3, kc b4-7, vc b0-3, wo c0-2, wo c3-4
            assert MODE == "bf16", "only the pure-bf16 path is maintained"
            with tc.tile_pool(name="pqkv", bufs=1, space="PSUM") as pqkv:
                q_ps = pqkv.tile([128, CW], F32, tag="q")
                kv_ps = pqkv.tile([128, KVW], F32, tag="kv")
                warm_ps = pqkv.tile([128, 128], F32, tag="warm")
                HKT = NKT // 2

                if skip_in_dma == "shadow":
                    shadow_t = {}
                    for nm, eng in (("sp", nc.sync), ("act", nc.scalar),
                                    ("pool", nc.gpsimd)):
                        shadow_t[id(eng)] = (
                            [cpool.tile([128, 4096], WT, name=f"sh_{nm}{j}")
                             for j in range(2)], [0])

                def _dma(eng, dst, src):
                    # skip_in_dma: 16-col stub transfer (keeps tile writes +
                    # dep structure, negligible bytes) for compute-only bench.
                    # "shadow": additionally stream the full bytes to scratch
                    # tiles on the same queue (measures DMA/compute resource
                    # contention with data deps removed).
                    if skip_in_dma:
                        di = eng.dma_start(dst[:, 0:16], src[:, 0:16])
                        if skip_in_dma == "shadow":
                            tiles, ctr = shadow_t[id(eng)]
                            w = src.shape[-1]
                            for o in range(0, w, 4096):
                                cw = min(4096, w - o)
                                t = tiles[ctr[0] % 2]
                                ctr[0] += 1
                                eng.dma_start(t[:, 0:cw], src[:, o:o + cw])
                        return di
                    return eng.dma_start(dst, src)

                wq_t = cpool.tile([128, NKT * CW], WT)
                xt_t = cpool.tile([128, D], WT)
                wkv_t = cpool.tile([128, NKT * KVW], WT)

                # SP stream: xt -> wqA -> wkv -> vc b4-7 -> wo c3-4
                _dma(nc.sync, xt_t[:], xt_d[:])
                QCH = HKT * CW  # 16 k-tiles = 2MB per wq half
                _dma(nc.sync, wq_t[:, 0:QCH], wq_d[:, 0:QCH])
                sp_last_in = _dma(nc.sync, wkv_t[:], wkv_d[:])
                # Pool stream: consts -> wqB -> kc -> vc b0-3 -> wo c0-2, c5-7
                _dma(nc.gpsimd, rotc[:], rotc_d[:])
                wqb_di = _dma(nc.gpsimd, wq_t[:, QCH:2 * QCH],
                              wq_d[:, QCH:2 * QCH])
                act_prev = None
                if ncache:
                    HKC = B // 2 * tpb * KBW
                    _dma(nc.gpsimd, kc_all[:, 0:HKC], keyst_d[:, 0:HKC])
                    _dma(nc.gpsimd, kc_all[:, HKC:2 * HKC],
                         keyst_d[:, HKC:2 * HKC])
                    HW_ = (B // 2) * tpb * VBW
                    _dma(nc.gpsimd, vc_half[0][:], vals_d[:, 0:HW_])
                    sp_last_in = _dma(nc.sync, vc_half[1][:],
                                      vals_d[:, HW_:2 * HW_])
                    kv_last_inst = sp_last_in
                # wo stream: c0-2 + c5-7 on Pool, c3-4 on SP
                WOW = HQ * 512
                WO_GROUPS = [[0, 1, 2], [3, 4], [5, 6, 7]]
                WO_ENG = [nc.gpsimd, nc.sync, nc.gpsimd]
                wo_tiles = {}   # chunk n -> (tile, col base within tile)
                for g, chunks in enumerate(WO_GROUPS):
                    gw = len(chunks) * WOW
                    wo_t = cpool.tile([128, gw], WT, name=f"woG{g}")
                    di = _dma(WO_ENG[g], wo_t[:],
                              wo_d[:, chunks[0] * WOW:chunks[0] * WOW + gw])
                    for ci, n in enumerate(chunks):
                        wo_tiles[n] = (wo_t, ci * WOW)
                    if WO_ENG[g] is nc.sync:
                        sp_last_in = di
                wo_last_dma = sp_last_in
                # ACT queue carries ONLY the 3 tail out DMAs: the scalar
                # engine's exp chain must not contend with input streams
                kv_out_dep = None

                # PE warm-up: ~3.5us of dummy matmuls during the initial DMA
                # wait so the HAM clock gate opens (1.2 -> 2.4 GHz) before
                # the q projection starts
                warm_last = None
                for w in range(24):
                    warm_last = nc.tensor.matmul(
                        warm_ps[:], ident[:], ident[:], start=True, stop=True)

                # q-proj matmul order follows chunk arrival: wqB1 and wqA1
                # stream first on their queues, then wqB2/wqA2
                ks = list(range(NKT))
                first = True
                for k in ks:
                    mm = nc.tensor.matmul(
                        q_ps[:], xt_t[:, k * 128:(k + 1) * 128],
                        wq_t[:, k * CW:(k + 1) * CW],
                        start=first, stop=(k == ks[-1]))
                    if first and warm_last is not None:
                        add_dep_helper(mm.ins, warm_last.ins,
                                       reason="warmup before q proj")
                    first = False

                # ------------ phase 2+3: rotary + transposes --------------
                # rot = t*crep ; rot_even -= t_odd*shalf ; rot_odd += t_even*shalf
                # split into q-segment / k-segment so the q path does not
                # wait for the kv projection
                with tc.tile_pool(name=f"rotp{_rep}", bufs=1) as rotp:
                    W = (HQ + 1) * HD  # 640 = 4 q heads + 1 k head
                    rot = rotp.tile([128, W], F32, tag="rot")
                    rotB = rotp.tile([128, W], BF, tag="rotB")
                    tmpa = rotp.tile([128, W // 2], F32, tag="tmpa")
                    tmpb = rotp.tile([128, W // 2], F32, tag="tmpb")
                    q_v = q_ps[:].rearrange("p (a two) -> p a two", two=2)
                    k_v = kv_ps[:, 0:HD].rearrange("p (a two) -> p a two", two=2)
                    rot_v = rot[:].rearrange("p (a two) -> p a two", two=2)
                    HCW = CW // 2
                    # q segment (DVE)
                    nc.vector.tensor_mul(rot[:, 0:CW], q_ps[:], crep[:, 0:CW])
                    nc.vector.tensor_mul(tmpa[:, 0:HCW], q_v[:, :, 1], shalf[:, 0:HCW])
                    nc.vector.tensor_mul(tmpb[:, 0:HCW], q_v[:, :, 0], shalf[:, 0:HCW])
                    nc.vector.tensor_sub(rot_v[:, 0:HCW, 0], rot_v[:, 0:HCW, 0], tmpa[:, 0:HCW])
                    nc.vector.tensor_add(rot_v[:, 0:HCW, 1], rot_v[:, 0:HCW, 1], tmpb[:, 0:HCW])
                    nc.vector.tensor_copy(rotB[:, 0:CW], rot[:, 0:CW])

                    vaugB = rotp.tile([128, HD], BF, tag="vaug")
                    # qtH is stored batch-major: column b*64 + h*16 + s, so the
                    # scores rhs is a contiguous (128, 64) slice per batch
                    qt_v = qtH[:].rearrange("p (b h s) -> p b h s", b=B, h=HQ)
                    vt = rotp.tile([128, 128], BF, tag="vt")
                    with tc.tile_pool(name=f"ptr{_rep}", bufs=2, space="PSUM") as ptr:
                        # q-head transposes first: the q path gates the whole
                        # attention pipeline
                        for h in range(HQ):
                            tp = ptr.tile([128, 128], BF, tag="tr", name=f"tr{h}")
                            nc.tensor.transpose(tp[:], rotB[:, h * 128:(h + 1) * 128], ident[:])
                            nc.vector.tensor_copy(
                                qt_v[:, :, h, :],
                                tp[:].rearrange("p (b s) -> p b s", b=B))

                        # kv projection (chases the wkv stream)
                        for k in range(NKT):
                            nc.tensor.matmul(
                                kv_ps[:], xt_t[:, k * 128:(k + 1) * 128],
                                wkv_t[:, k * KVW:(k + 1) * KVW],
                                start=(k == 0), stop=(k == NKT - 1))
                        # k segment rotary (DVE)
                        nc.vector.tensor_mul(rot[:, CW:W], kv_ps[:, 0:HD], crep[:, CW:W])
                        nc.vector.tensor_mul(tmpa[:, HCW:], k_v[:, :, 1], shalf[:, HCW:])
                        nc.vector.tensor_mul(tmpb[:, HCW:], k_v[:, :, 0], shalf[:, HCW:])
                        nc.vector.tensor_sub(rot_v[:, HCW:, 0], rot_v[:, HCW:, 0], tmpa[:, HCW:])
                        nc.vector.tensor_add(rot_v[:, HCW:, 1], rot_v[:, HCW:, 1], tmpb[:, HCW:])
                        nc.vector.tensor_copy(rotB[:, CW:W], rot[:, CW:W])

                        tp = ptr.tile([128, 128], BF, tag="tr", name="trk")
                        nc.tensor.transpose(tp[:], rotB[:, CW:W], ident[:])
                        nc.vector.tensor_copy(ktH[:], tp[:])
                        # new-v rebased to partition 0 via double transpose (no
                        # DMA: the sbuf->sbuf route queues behind ~20MB of
                        # loads and stalls PV(0) -> in-order PE for ~14us)
                        nc.vector.tensor_copy(vaugB[:], kv_ps[:, HD:KVW])
                        tpv = ptr.tile([128, 128], BF, tag="tr", name="trv")
                        nc.tensor.transpose(tpv[:], vaugB[:], ident[:])
                        nc.vector.tensor_copy(vt[:], tpv[:])  # (hd, tok)
                        for b in range(B):
                            tq = ptr.tile([S, 128], BF, tag="trb", name=f"trb{b}")
                            nc.tensor.transpose(tq[:], vt[:, b * S:(b + 1) * S], ident[:])
                            nc.vector.tensor_copy(
                                vrebB[:, b * (HD + 1):b * (HD + 1) + HD], tq[:])
                            nc.vector.memset(
                                vrebB[:, b * (HD + 1) + HD:(b + 1) * (HD + 1)], 1.0)

            # ---------------- phase 4: attention per batch ----------------
            if True:
              with (
                  tc.tile_pool(name=f"expp{_rep}", bufs=2) as expp,
                  tc.tile_pool(name=f"psc{_rep}", bufs=2, space="PSUM") as psc,
                  tc.tile_pool(name=f"pctx{_rep}", bufs=2, space="PSUM") as pctx,
                  tc.tile_pool(name=f"pctr{_rep}", bufs=2, space="PSUM") as pctr,
              ):
                  st = {}

                  def do_pv(i):
                      # PV + normalize for batch i (exp already done)
                      exHi = st[i]["exH"]
                      vci, vbase = st[i]["vc"]
                      VB = HD + 1
                      ctx = pctx.tile([QW, HD + 1], F32, tag="ctx", name=f"ctx{i}")
                      for j, cs in enumerate(CS):
                          nc.tensor.matmul(
                              ctx[:], exHi[0:cs, j * QW:(j + 1) * QW],
                              vci[0:cs, vbase + j * VB:vbase + (j + 1) * VB],
                              start=(j == 0), stop=False)
                      nc.tensor.matmul(
                          ctx[:], exHi[0:S, ncache * QW:NCHUNK * QW],
                          vrebB[:, i * (HD + 1):(i + 1) * (HD + 1)],
                          start=(ncache == 0), stop=True)
                      rc = expp.tile([QW, 1], F32, tag="rc", name=f"rc{i}")
                      nc.vector.reciprocal(rc[:], ctx[:, HD:HD + 1])
                      cn = expp.tile([QW, HD], BF, tag="cn", name=f"cn{i}")
                      nc.vector.tensor_scalar_mul(cn[:], ctx[:, 0:HD], rc[:])
                      st[i]["cn"] = cn

                  def do_tr(i):
                      # transpose + scatter into ctxtB for batch i
                      ct = pctr.tile([128, QW], BF, tag="ct", name=f"ct{i}")
                      nc.tensor.transpose(ct[:], st[i]["cn"][:], ident[0:QW, 0:QW])
                      nc.vector.tensor_copy(
                          ctxtB[:].rearrange("p (h b s) -> p h b s", h=HQ, b=B)[:, :, i, :],
                          ct[:].rearrange("p (h s) -> p h s", h=HQ))

                  for b in range(B):
                      if ncache:
                          kbase = b * tpb * KBW
                          vc_t = (vc_half[b // (B // 2)],
                                  (b % (B // 2)) * tpb * VBW)
                      else:
                          kbase = vc_t = None
                      qbH = qtH[:, b * QW:(b + 1) * QW]

                      sc = psc.tile([128, SCW], F32, tag="sc", name=f"sc{b}")
                      # pre-fill columns of partial chunks so garbage partitions
                      # exp() to 0; the matmuls below overwrite the valid rows
                      nc.vector.memset(sc[:, ncache * QW:NCHUNK * QW], -1e30)
                      for j, cs in enumerate(CS):
                          if cs < 128:
                              nc.vector.memset(sc[:, j * QW:(j + 1) * QW], -1e30)
                      for j, cs in enumerate(CS):
                          nc.tensor.matmul(
                              sc[0:cs, j * QW:(j + 1) * QW],
                              kc_all[:, kbase + j * 128:kbase + j * 128 + cs],
                              qbH, start=True, stop=True)
                      # new-kv chunk
                      nc.tensor.matmul(
                          sc[0:S, ncache * QW:NCHUNK * QW],
                          ktH[:, b * S:(b + 1) * S], qbH,
                          start=True, stop=True)

                      # software pipeline: PE stays busy with prior batches'
                      # PV/transpose while ACT runs this batch's exp
                      if b >= 1:
                          do_pv(b - 1)
                      if b >= 2:
                          do_tr(b - 2)

                      exH = expp.tile([128, SCW], BF, tag="exH", name=f"exH{b}")
                      nc.scalar.activation(exH[:], sc[:], mybir.ActivationFunctionType.Exp,
                                           scale=scale)
                      st[b] = dict(exH=exH, vc=vc_t)

                  do_pv(B - 1)
                  do_tr(B - 2)
                  do_tr(B - 1)

              # ---------------- phase 5: wo projection ----------------
              # wo is chunked by OUTPUT columns: each 1MB chunk holds all 4
              # h-blocks for one 512-col output slice, so its psum completes
              # and streams out immediately
              with tc.tile_pool(name=f"pwo{_rep}", bufs=3, space="PSUM") as pwo:
                  # chunk order follows wo stream arrival (c5-7 land first on
                  # ACT, then Pool's c0-2, c3-4); out rides in 3 pieces so
                  # each queue pays one ~1us dma_start and the final piece
                  # (out4, 128KB) keeps the tail short.
                  ORDER = [0, 1, 2, 3, 4, 5, 6, 7]
                  sp_out_dep = None
                  for n in ORDER:
                      wo_t, wbase = wo_tiles[n]
                      op_t = pwo.tile([128, 512], F32, tag="o", name=f"o{n}")
                      for h in range(HQ):
                          nc.tensor.matmul(
                              op_t[:], ctxtB[:, h * 128:(h + 1) * 128],
                              wo_t[:, wbase + h * 512:wbase + (h + 1) * 512],
                              start=(h == 0), stop=(h == HQ - 1))
                      nc.vector.tensor_copy(otile[:, n * 512:(n + 1) * 512], op_t[:])
                      if n == 3:    # out c0-3 on ACT (DMA-idle queue)
                          odi = nc.scalar.dma_start(
                              out_d[:, 0 * 512:4 * 512],
                              otile[:, 0 * 512:4 * 512])
                          sp_out_dep = odi
                      elif n == 6:  # out c4-6
                          odi = nc.scalar.dma_start(
                              out_d[:, 4 * 512:7 * 512],
                              otile[:, 4 * 512:7 * 512])
                          if sp_out_dep is not None:
                              add_dep_helper(odi.ins, sp_out_dep.ins,
                                             reason="act out order")
                          sp_out_dep = odi
                      elif n == 7:  # final small piece
                          odi = nc.scalar.dma_start(
                              out_d[:, 7 * 512:8 * 512],
                              otile[:, 7 * 512:8 * 512])
                          if sp_out_dep is not None:
                              add_dep_helper(odi.ins, sp_out_dep.ins,
                                             reason="act out order")

    nc.compile()
    return nc


def _prep_host(x, wq, wk, wv, wo, cos, sin, cache_k, cache_v, L):
    """Pack full inputs into per-core DMA-friendly slabs."""
    K_BF16C = USE_BF16C
    B, S, _ = x.shape
    T = B * S
    nfull, rem = divmod(L, 128)
    tpb = nfull + (1 if rem else 0)

    f = np.float32
    bf = mybir.dt.np(mybir.dt.bfloat16)

    def hilo(a):
        # interleave per-chunk [hi | lo] along axis 1 at chunk granularity is
        # done by the callers; here: full-width hi/lo halves
        hi = a.astype(bf)
        lo = (a - hi.astype(f)).astype(bf)
        return hi, lo

    x_flat = np.ascontiguousarray(np.asarray(x, f).reshape(T, D))
    xt = np.ascontiguousarray(
        x_flat.reshape(T, D // 128, 128).transpose(2, 1, 0).reshape(128, -1))

    cs_ = np.asarray(cos, f)[L:L + S]
    sn_ = np.asarray(sin, f)[L:L + S]
    crep = np.tile(np.tile(np.repeat(cs_, 2, axis=1), (B, 1)), (1, HQ + 1))
    shalf = np.tile(np.tile(sn_, (B, 1)), (1, HQ + 1))
    ident = np.eye(128, dtype=f)
    rotc = np.ascontiguousarray(
        np.concatenate([crep, shalf, ident], axis=1)
        .astype(bf if MODE == "bf16" else f))

    wq = np.asarray(wq, f)
    wk = np.asarray(wk, f)
    wv = np.asarray(wv, f)
    wo = np.asarray(wo, f)
    cache_k = np.asarray(cache_k, f)
    cache_v = np.asarray(cache_v, f)

    if K_BF16C:
        xth, xtl = hilo(xt)
        xt = np.ascontiguousarray(np.concatenate(
            [np.concatenate([xth[:, c * 1024:(c + 1) * 1024],
                             xtl[:, c * 1024:(c + 1) * 1024]], axis=1)
             for c in range(4)], axis=1))
    elif MODE == "bf16":
        xt = np.ascontiguousarray(xt.astype(bf))
    shared = dict(xt=xt, rotc=rotc)
    in_maps = []
    for c in range(NCORES):
        wq_c = wq[:, c * CW:(c + 1) * CW]
        wq_l = np.ascontiguousarray(
            wq_c.reshape(D // 128, 128, CW).transpose(1, 0, 2).reshape(128, -1))
        wkv_c = np.concatenate(
            [wk[:, c * HD:(c + 1) * HD], wv[:, c * HD:(c + 1) * HD]], axis=1)
        wkv_l = np.ascontiguousarray(
            wkv_c.reshape(D // 128, 128, KVW).transpose(1, 0, 2).reshape(128, -1))
        wo_c = wo[c * CW:(c + 1) * CW, :]
        # [p, n*2048 + h*512 + nn] = wo_c[h*128 + p, n*512 + nn]
        wo_l = np.ascontiguousarray(
            wo_c.reshape(HQ, 128, D // 512, 512).transpose(1, 2, 0, 3).reshape(128, -1))
        if K_BF16C:
            def chunked_hilo(a, nch):
                w = a.shape[1] // nch
                hi, lo = hilo(a)
                return np.ascontiguousarray(np.concatenate(
                    [np.concatenate([hi[:, i * w:(i + 1) * w],
                                     lo[:, i * w:(i + 1) * w]], axis=1)
                     for i in range(nch)], axis=1))
            wq_l = chunked_hilo(wq_l, 4)    # 4 weight chunks
            wkv_l = chunked_hilo(wkv_l, 4)
            wo_l = chunked_hilo(wo_l, 8)    # 8 column chunks
        elif MODE == "bf16":
            wq_l = np.ascontiguousarray(wq_l.astype(bf))
            wkv_l = np.ascontiguousarray(wkv_l.astype(bf))
            wo_l = np.ascontiguousarray(wo_l.astype(bf))
        m = dict(shared, wq=wq_l, wkv=wkv_l, wo=wo_l)
        if tpb:
            kpad = np.zeros((B, tpb * 128, 128), f)
            kpad[:, :L] = cache_k[:, :L, c, :]
            kl = np.ascontiguousarray(
                kpad.reshape(B, tpb, 128, 128).transpose(3, 0, 1, 2).reshape(128, -1))
            if K_BF16C:
                kh, klo = hilo(kl)
                nchk = B * tpb
                kl = np.ascontiguousarray(np.concatenate(
                    [np.concatenate([kh[:, i*128:(i+1)*128],
                                     klo[:, i*128:(i+1)*128]], axis=1)
                     for i in range(nchk)], axis=1))
            elif MODE == "bf16":
                kl = np.ascontiguousarray(kl.astype(bf))
            m["keyst"] = kl
            vpad = np.zeros((B, tpb * 128, HD + 1), f)
            vpad[:, :L, :HD] = cache_v[:, :L, c, :]
            vpad[:, :L, HD] = 1.0
            vl = np.ascontiguousarray(
                vpad.reshape(B, tpb, 128, HD + 1).transpose(2, 0, 1, 3).reshape(128, -1))
            if K_BF16C:
                # per (b,j) chunk: [hi(129) | lo(129)] adjacent
                vh, vlo = hilo(vl)
                nchv = B * tpb
                vl = np.ascontiguousarray(np.concatenate(
                    [np.concatenate([vh[:, i*(HD+1):(i+1)*(HD+1)],
                                     vlo[:, i*(HD+1):(i+1)*(HD+1)]], axis=1)
                     for i in range(nchv)], axis=1))
            elif MODE == "bf16":
                vl = np.ascontiguousarray(vl.astype(bf))
            m["vals"] = vl
        in_maps.append(m)
    return in_maps


def kernel(x, wq, wk, wv, wo, cos, sin, cache_k, cache_v, start_pos):
    global _LAST
    B, S, _ = x.shape
    L = int(start_pos)

    key = (L, B, S)
    if key not in _BUILD_CACHE:
        _BUILD_CACHE[key] = _build(L, B, S)
    nc = _BUILD_CACHE[key]

    in_maps = _prep_host(x, wq, wk, wv, wo, cos, sin, cache_k, cache_v, L)
    res = bass_utils.run_bass_kernel_spmd(
        nc, in_maps, core_ids=list(range(NCORES)),
        trace=TRACE or bool(os.environ.get("BASS_TRACE")))
    _LAST = res
    out = np.zeros((B * S, D), np.float32)
    for r in res.results:
        out += np.asarray(r["out"], np.float32)
    return out.reshape(B, S, D)

